# revision 24
# baseline (speedup 1.0000x reference)
"""Trainium2 Bass kernel for nn_CapsuleLayer (dynamic routing).

Problem:  u_hat = einsum('bri,crio->cbro', x, W);  3 routing iterations
          (softmax over R, weighted sum, squash, agreement update).
Shapes:   x [256, 1152, 8] f32, W [10, 1152, 8, 16] f32 ->
          out [10, 256, 1, 1, 16] f32.

Strategy (8 NeuronCores, data-parallel over batch, B_loc = 32/core):
  * all W layouts fp16, loaded once; u_hat never materialized.
  * it0 s-pass: stationary = x chunk [128,32] (cheap LDWEIGHTS),
    moving = wfrp [128,320] (all classes, 32-col padded slots);
    squash via tensor_reduce + DVE 32x32 stream transpose.
  * rounds are WAVE-PIPELINED: the 10 classes split into 3 waves
    (4/4/2); each wave's chain agreement -> softmax -> s-pass ->
    squash is independent, so wave w's DVE-heavy agreement overlaps
    wave w+1's PE-heavy matmuls.
  * s-pass it>0: stationary = wfrp wave group [128,<=128] (each class
    in a 32-col slot so PSUM diagonal blocks are 32-aligned), moving =
    y = cw*x fp16; PSUM memset + start=False (start zeroes a whole
    PSUM bank).
  * agreement: block-diagonal [128,128] fp16 stationary per wave (v
    embedded on device), moving = resident wt2 [128,512] chunks;
    i-reduction via DVE tensor_reduce (2 of 3 chunks) and GPSIMD
    tree (1 of 3) to balance engines.
Logit-path partial sums stay fp32 (products only rounded to fp16).
"""

import sys
from contextlib import ExitStack

import numpy as np

sys.path.insert(0, "/opt/trn_rl_repo")

import concourse.bacc as bacc
import concourse.bass as bass
import concourse.mybir as mybir
import concourse.tile as tile
from concourse.bass_utils import run_bass_kernel_spmd

F32 = mybir.dt.float32
F16 = mybir.dt.float16
MUL = mybir.AluOpType.mult
ADD = mybir.AluOpType.add
AXX = mybir.AxisListType.X

B, R, I, C, O = 256, 1152, 8, 10, 16
NC = 8
BL = B // NC          # 32 batch per core
Q = R // 128          # 9 r-blocks of 128
CO = C * O            # 160
CB = C * BL           # 320
RI = R * I            # 9216
PW = 320              # padded stationary width: 10 classes x 32-col slots
EPS = 1e-7
GCH = 1024            # agreement chunk (elements of (r,i)) = 128 r
NG = RI // GCH        # 9 chunks
W3 = 3                # class waves: w0 = c0-3, w1 = c4-7, w2 = c8-9


def wave_ncls(w):
    return 4 if w < 2 else 2


def build_nc(debug=False):
    nc = bacc.Bacc("TRN2", target_bir_lowering=False, debug=debug)

    xtr_d = nc.declare_dram_parameter("xtr", [128, Q, I, BL], F16, isOutput=False)
    wfrp_d = nc.declare_dram_parameter("wfrp", [128, Q, I, PW], F16, isOutput=False)
    w2a_d = nc.declare_dram_parameter("w2a", [128, 2, RI], F16, isOutput=False)
    w2b_d = nc.declare_dram_parameter("w2b", [64, RI], F16, isOutput=False)
    xrep_d = nc.declare_dram_parameter("xrep", [128, RI], F16, isOutput=False)
    ident_d = nc.declare_dram_parameter("ident", [128, 128], F32, isOutput=False)
    out_d = nc.declare_dram_parameter("out", [C, O, BL], F32, isOutput=True)

    with tile.TileContext(nc) as tc, ExitStack() as ctx:
        res = ctx.enter_context(tc.tile_pool(name="res", bufs=1))
        yp = ctx.enter_context(tc.tile_pool(name="yp", bufs=2))
        gmp = ctx.enter_context(tc.tile_pool(name="gmp", bufs=2))
        trp = ctx.enter_context(tc.tile_pool(name="trp", bufs=2))
        cwp = ctx.enter_context(tc.tile_pool(name="cwp", bufs=1))
        smp = ctx.enter_context(tc.tile_pool(name="smp", bufs=1))
        psS = ctx.enter_context(
            tc.tile_pool(name="psS", bufs=2, space=bass.MemorySpace.PSUM)
        )
        psG = ctx.enter_context(
            tc.tile_pool(name="psG", bufs=2, space=bass.MemorySpace.PSUM)
        )
        psT = ctx.enter_context(
            tc.tile_pool(name="psT", bufs=1, space=bass.MemorySpace.PSUM)
        )
        psN = ctx.enter_context(
            tc.tile_pool(name="psN", bufs=1, space=bass.MemorySpace.PSUM)
        )

        # ---- resident tensors -------------------------------------
        xtr = res.tile([128, Q, I, BL], F16)
        wfrp = res.tile([128, Q, I, PW], F16)
        w2a = res.tile([128, 2, RI], F16)
        w2b = res.tile([64, RI], F16)
        xrep = res.tile([128, RI], F16)
        ident = res.tile([128, 128], F32)
        L = res.tile([128, W3, R], F32)
        S = res.tile([128, W3, 128], F16)    # agreement stationaries
        cwT = res.tile([128, Q, CB], F16)    # softmax weights, [rr, q, 32c+b]
        sps = res.tile([16, C, BL], F32)     # gathered s, [o, c, b]
        v_sb = res.tile([16, C, BL], F32)    # squash output, [o, c, b]
        Z = res.tile([128, W3], F32)
        Zi = res.tile([128, W3], F32)
        ones16 = res.tile([16, 1], F32)

        # ---- input DMAs (ordered to overlap with it0 compute) -----
        nc.sync.dma_start(ident[:], ident_d[:])
        nc.sync.dma_start(xtr[:], xtr_d[:])
        for q in range(Q):
            nc.sync.dma_start(wfrp[:, q], wfrp_d[:, q])
        nc.sync.dma_start(xrep[:], xrep_d[:])
        SEG = RI // 3
        for w in range(2):
            for s3 in range(3):
                nc.sync.dma_start(
                    w2a[:, w, s3 * SEG : (s3 + 1) * SEG],
                    w2a_d[:, w, s3 * SEG : (s3 + 1) * SEG],
                )
        for s3 in range(3):
            nc.sync.dma_start(
                w2b[:, s3 * SEG : (s3 + 1) * SEG],
                w2b_d[:, s3 * SEG : (s3 + 1) * SEG],
            )
        nc.vector.memset(S[:], 0.0)
        nc.vector.memset(ones16[:], 1.0)

        # ---------------------------------------------------------------
        def squash_chain(sn, f, n, it0=False):
            """f = squash scale from sn = |s|^2 (on [p, n] tiles)."""
            u1 = smp.tile(list(sn.shape), F32, tag="u1")
            u2 = smp.tile(list(sn.shape), F32, tag="u2")
            u3 = smp.tile(list(sn.shape), F32, tag="u3")
            if it0:
                nc.vector.tensor_scalar_mul(sn[:, :n], sn[:, :n], 1.0 / (R * R))
            nc.vector.tensor_scalar_add(u1[:, :n], sn[:, :n], EPS)
            nc.scalar.activation(
                u2[:, :n], u1[:, :n], mybir.ActivationFunctionType.Sqrt
            )
            nc.vector.tensor_scalar_add(u3[:, :n], sn[:, :n], 1.0)
            nc.vector.tensor_tensor(u1[:, :n], u2[:, :n], u3[:, :n], MUL)
            nc.vector.reciprocal(u2[:, :n], u1[:, :n])
            nc.vector.tensor_tensor(f[:, :n], sn[:, :n], u2[:, :n], MUL)
            if it0:
                nc.vector.tensor_scalar_mul(f[:, :n], f[:, :n], 1.0 / R)

        # ---------------------------------------------------------------
        # it0: s = sum over (r,i) of W * x (uniform weights folded into
        # the squash scalars).  One matmul per (q,i): 320-col moving.
        ps0 = psS.tile([128, PW], F32, tag="spsB")
        for q in range(Q):
            for i in range(I):
                nc.tensor.matmul(
                    ps0[:BL, :],
                    xtr[:, q, i, :],
                    wfrp[:, q, i, :],
                    start=(q == 0 and i == 0),
                    stop=(q == Q - 1 and i == I - 1),
                )
        sq0 = smp.tile([BL, C, 32], F32, tag="sq0")
        nc.scalar.activation(
            sq0[:],
            ps0[:BL, :].rearrange("b (c s) -> b c s", s=32),
            mybir.ActivationFunctionType.Square,
        )
        sn0 = smp.tile([BL, C], F32, tag="sn0")
        nc.vector.tensor_reduce(sn0[:], sq0[:], AXX, ADD)
        f0 = smp.tile([BL, C], F32, tag="f0")
        squash_chain(sn0, f0, C, it0=True)
        v0 = smp.tile([BL, PW], F32, tag="v0")
        nc.vector.tensor_tensor(
            v0[:].rearrange("b (c s) -> b c s", s=32),
            ps0[:BL, :].rearrange("b (c s) -> b c s", s=32),
            f0[:].unsqueeze(2).broadcast_to([BL, C, 32]),
            MUL,
        )
        vT = smp.tile([BL, PW], F32, tag="vT")
        nc.vector.transpose(vT[:], v0[:])    # 32x32 blocks: [o, 32c+b]
        for c in range(C):
            w, k = (c // 4, c % 4) if c < 8 else (2, c - 8)
            nc.scalar.copy(
                S[32 * k : 32 * k + 16, w, 32 * k : 32 * k + 32],
                vT[0:16, 32 * c : 32 * c + 32],
            )

        # ---------------------------------------------------------------
        def agreement_wave(w, acc):
            """L[p=(k,b), w, r] (+)= sum_{i,o} v*W*x for wave w."""
            npart = 128 if w < 2 else 64
            for n0 in range(NG):
                off = n0 * GCH
                gps = psG.tile([128, GCH], F32, tag="gps")
                for sub in range(0, GCH, 512):
                    if w < 2:
                        nc.tensor.matmul(
                            gps[:, sub : sub + 512],
                            S[:, w, :],
                            w2a[:, w, off + sub : off + sub + 512],
                            start=True,
                            stop=True,
                        )
                    else:
                        nc.tensor.matmul(
                            gps[:64, sub : sub + 512],
                            S[:64, 2, :64],
                            w2b[:, off + sub : off + sub + 512],
                            start=True,
                            stop=True,
                        )
                Lo = L[:npart, w, 128 * n0 : 128 * (n0 + 1)]
                if n0 % 2 == 1:
                    # GPSIMD unit (cannot read PSUM): scalar drains the
                    # bank to fp16 SBUF, GPSIMD does multiply + tree.
                    gsb = gmp.tile([128, GCH], F16, tag="gsb")
                    nc.scalar.copy(gsb[:npart], gps[:npart])
                    gm = gmp.tile([128, GCH], F16, tag="gmg")
                    nc.gpsimd.tensor_tensor(
                        gm[:npart], gsb[:npart], xrep[:npart, off : off + GCH], MUL
                    )
                    gmv = gm.rearrange("p (r i) -> p r i", i=I)
                    l1 = trp.tile([128, GCH // 2], F16, tag="l1")
                    l2 = trp.tile([128, GCH // 4], F16, tag="l2")
                    l1v = l1.rearrange("p (r i) -> p r i", i=4)
                    l2v = l2.rearrange("p (r i) -> p r i", i=2)
                    nc.gpsimd.tensor_tensor(
                        l1v[:npart], gmv[:npart, :, 0:4], gmv[:npart, :, 4:8], ADD
                    )
                    nc.gpsimd.tensor_tensor(
                        l2v[:npart], l1v[:npart, :, 0:2], l1v[:npart, :, 2:4], ADD
                    )
                    if not acc:
                        nc.gpsimd.tensor_tensor(
                            Lo, l2v[:npart, :, 0], l2v[:npart, :, 1], ADD
                        )
                    else:
                        a = trp.tile([128, GCH // 8], F32, tag="a")
                        nc.gpsimd.tensor_tensor(
                            a[:npart], l2v[:npart, :, 0], l2v[:npart, :, 1], ADD
                        )
                        nc.gpsimd.tensor_tensor(Lo, Lo, a[:npart], ADD)
                else:
                    gm = gmp.tile([128, GCH], F16, tag="gm")
                    nc.vector.tensor_tensor(
                        gm[:npart], gps[:npart], xrep[:npart, off : off + GCH], MUL
                    )
                    gmv = gm[:npart].rearrange("p (r i) -> p r i", i=I)
                    if not acc:
                        nc.vector.tensor_reduce(Lo, gmv, AXX, ADD)
                    else:
                        red = trp.tile([128, 128], F32, tag="red")
                        nc.vector.tensor_reduce(red[:npart], gmv, AXX, ADD)
                        nc.vector.tensor_tensor(Lo, Lo, red[:npart], ADD)

        # ---------------------------------------------------------------
        def softmax_wave(w):
            """cw = softmax_r(L[.,w,.]); transposed fp16 into cwT."""
            npart = 128 if w < 2 else 64
            cwv = cwp.tile([128, R], F32, tag=f"cw{w}")
            nc.scalar.activation(
                cwv[:npart],
                L[:npart, w, :],
                mybir.ActivationFunctionType.Exp,
                accum_out=Z[:npart, w : w + 1],
            )
            nc.vector.reciprocal(Zi[:npart, w : w + 1], Z[:npart, w : w + 1])
            nc.vector.tensor_scalar_mul(
                cwv[:npart], cwv[:npart], Zi[:npart, w : w + 1]
            )
            for q in range(Q):
                tps = psT.tile([128, 128], F32, tag="tps")
                nc.tensor.transpose(
                    tps[:, :npart],
                    cwv[:npart, 128 * q : 128 * (q + 1)],
                    ident[:npart, :npart],
                )
                nc.scalar.copy(cwT[:, q, 128 * w : 128 * w + npart], tps[:, :npart])

        # ---------------------------------------------------------------
        def spass_wave(w):
            """s[4w+cc, o, b] = sum_{r,i} W * cw * x for wave w."""
            ncls = wave_ncls(w)
            nw = 32 * ncls
            spsB = psS.tile([128, PW], F32, tag="spsB")
            for q in range(Q):
                y_q = yp.tile([128, I, 128], F16, tag="y")
                eng = nc.gpsimd if q % 3 == 2 else nc.vector
                eng.tensor_tensor(
                    y_q[:, :, :nw].rearrange("p i (c b) -> p i c b", b=BL),
                    xtr[:, q, :, :]
                    .unsqueeze(2)
                    .broadcast_to([128, I, ncls, BL]),
                    cwT[:, q, 128 * w : 128 * w + nw]
                    .rearrange("p (c b) -> p c b", b=BL)
                    .unsqueeze(1)
                    .broadcast_to([128, I, ncls, BL]),
                    MUL,
                )
                for i in range(I):
                    nc.tensor.matmul(
                        spsB[:nw, :nw],
                        wfrp[:, q, i, 128 * w : 128 * w + nw],
                        y_q[:, i, :nw],
                        start=(q == 0 and i == 0),
                        stop=(q == Q - 1 and i == I - 1),
                    )
            for cc in range(ncls):
                nc.scalar.copy(
                    sps[:, 4 * w + cc, :],
                    spsB[32 * cc : 32 * cc + 16, 32 * cc : 32 * cc + 32],
                )

        def squash_group(c0, ncg, fill):
            """v_sb[:, c0:c0+ncg, :] = squash(s) over o (+ S fill)."""
            n = 32 * ncg
            spw = sps[:, c0 : c0 + ncg, :].rearrange("o c b -> o (c b)")
            sq = smp.tile([16, CB], F32, tag="sq")
            nc.vector.tensor_tensor(sq[:, :n], spw, spw, MUL)
            snps = psN.tile([1, CB], F32, tag="snps")
            nc.tensor.matmul(
                snps[:, :n], ones16[:], sq[:, :n], start=True, stop=True
            )
            f = smp.tile([1, CB], F32, tag="f")
            squash_chain(snps, f, n)
            frep = smp.tile([16, CB], F32, tag="frep")
            nc.gpsimd.partition_broadcast(frep[:, :n], f[:, :n])
            nc.vector.tensor_tensor(
                v_sb[:, c0 : c0 + ncg, :].rearrange("o c b -> o (c b)"),
                spw,
                frep[:, :n],
                MUL,
            )
            if fill:
                for c in range(c0, c0 + ncg):
                    w, k = (c // 4, c % 4) if c < 8 else (2, c - 8)
                    nc.scalar.copy(
                        S[32 * k : 32 * k + 16, w, 32 * k : 32 * k + 32],
                        v_sb[:, c, :],
                    )

        # =========================== flow ==============================
        for it in (1, 2):
            for w in range(W3):
                agreement_wave(w, acc=(it == 2))
                softmax_wave(w)
                spass_wave(w)
                squash_group(4 * w, wave_ncls(w), fill=(it < 2))

        nc.sync.dma_start(out_d[:].rearrange("c o b -> o c b"), v_sb[:])

    nc.compile()
    return nc


# =================== host-side prep / entry point =====================

def _prep_shared(W):
    """Per-problem constant tensors (replicated on every core)."""
    W = np.ascontiguousarray(W, np.float32)
    # wfrp[rr, q, i, 32c + o] = W[c, 128q+rr, i, o] (16-col zero pad per class)
    wfrp = np.zeros((128, Q, I, PW), np.float16)
    wr = W.reshape(C, Q, 128, I, O).transpose(2, 1, 3, 0, 4)  # [rr,q,i,c,o]
    for c in range(C):
        wfrp[:, :, :, 32 * c : 32 * c + O] = wr[:, :, :, c]
    # w2a[32k+o, w, 8r+i] = W[4w+k, r, i, o];  w2b[32k+o, 8r+i] = W[8+k,...]
    w2a = np.zeros((128, 2, RI), np.float16)
    for w in range(2):
        for k in range(4):
            w2a[32 * k : 32 * k + 16, w, :] = (
                W[4 * w + k].transpose(2, 0, 1).reshape(O, RI)
            )
    w2b = np.zeros((64, RI), np.float16)
    for k in range(2):
        w2b[32 * k : 32 * k + 16, :] = W[8 + k].transpose(2, 0, 1).reshape(O, RI)
    ident = np.eye(128, dtype=np.float32)
    return wfrp, w2a, w2b, ident


def _prep_core(x_shard):
    """Per-core tensors for one 32-batch shard: xtr and xrep."""
    xs = np.ascontiguousarray(x_shard, np.float32)       # [32, 1152, 8]
    xtr = np.ascontiguousarray(
        xs.reshape(BL, Q, 128, I).transpose(2, 1, 3, 0)
    ).astype(np.float16)                                  # [128, Q, I, 32]
    flat = xs.reshape(BL, RI)                             # [b, 8r+i]
    xrep = np.ascontiguousarray(
        flat[np.arange(128) % BL].astype(np.float16)
    )                                                     # [128, RI]
    return xtr, xrep


def prep_in_maps(x, W):
    wfrp, w2a, w2b, ident = _prep_shared(W)
    in_maps = []
    for m in range(NC):
        xtr, xrep = _prep_core(x[m * BL : (m + 1) * BL])
        in_maps.append(
            {
                "xtr": xtr,
                "wfrp": wfrp,
                "w2a": w2a,
                "w2b": w2b,
                "xrep": xrep,
                "ident": ident,
            }
        )
    return in_maps


_NC_CACHE = {}


def kernel(x, W):
    x = np.asarray(x, np.float32)
    W = np.asarray(W, np.float32)
    if "nc" not in _NC_CACHE:
        _NC_CACHE["nc"] = build_nc()
    nc = _NC_CACHE["nc"]

    res = run_bass_kernel_spmd(nc, prep_in_maps(x, W), list(range(NC)))
    out = np.empty((C, B, 1, 1, O), np.float32)
    for m in range(NC):
        o = res.results[m]["out"]                         # [C, O, BL]
        out[:, m * BL : (m + 1) * BL, 0, 0, :] = np.asarray(o).transpose(0, 2, 1)
    return out


if __name__ == "__main__":
    d = np.load("/root/problem/ref_data.npz")
    got = kernel(d["x"], d["W"])
    exp = d["expected"]
    err = np.abs(got - exp).max() / np.abs(exp).max()
    print("Relative error:", err)


# revision 25
# speedup vs baseline: 1.1662x; 1.1662x over previous
"""Trainium2 Bass kernel for nn_CapsuleLayer (dynamic routing).

Problem:  u_hat = einsum('bri,crio->cbro', x, W);  3 routing iterations
          (softmax over R, weighted sum, squash, agreement update).
Shapes:   x [256, 1152, 8] f32, W [10, 1152, 8, 16] f32 ->
          out [10, 256, 1, 1, 16] f32.

Strategy (8 NeuronCores, data-parallel over batch, B_loc = 32/core):
  * all W layouts fp16, loaded once; u_hat never materialized.
  * it0 s-pass: stationary = x chunk [128,32] (cheap LDWEIGHTS),
    moving = wfrp [128,320] (all classes, 32-col padded slots);
    squash via tensor_reduce + DVE 32x32 stream transpose.
  * rounds are WAVE-PIPELINED: the 10 classes split into 3 waves
    (4/4/2); each wave's chain agreement -> softmax -> s-pass ->
    squash is independent, so wave w's DVE-heavy agreement overlaps
    wave w+1's PE-heavy matmuls.
  * s-pass it>0: stationary = wfrp wave group [128,<=128] (each class
    in a 32-col slot so PSUM diagonal blocks are 32-aligned), moving =
    y = cw*x fp16; PSUM memset + start=False (start zeroes a whole
    PSUM bank).
  * agreement: block-diagonal [128,128] fp16 stationary per wave (v
    embedded on device), moving = resident wt2 [128,512] chunks;
    i-reduction via DVE tensor_reduce (2 of 3 chunks) and GPSIMD
    tree (1 of 3) to balance engines.
Logit-path partial sums stay fp32 (products only rounded to fp16).
"""

import sys
from contextlib import ExitStack

import numpy as np

sys.path.insert(0, "/opt/trn_rl_repo")

import concourse.bacc as bacc
import concourse.bass as bass
import concourse.mybir as mybir
import concourse.tile as tile
from concourse.bass_utils import run_bass_kernel_spmd

F32 = mybir.dt.float32
F16 = mybir.dt.float16
MUL = mybir.AluOpType.mult
ADD = mybir.AluOpType.add
AXX = mybir.AxisListType.X

B, R, I, C, O = 256, 1152, 8, 10, 16
NC = 8
BL = B // NC          # 32 batch per core
Q = R // 128          # 9 r-blocks of 128
CO = C * O            # 160
CB = C * BL           # 320
RI = R * I            # 9216
PW = 320              # padded stationary width: 10 classes x 32-col slots
EPS = 1e-7
GCH = 1024            # agreement chunk (elements of (r,i)) = 128 r
NG = RI // GCH        # 9 chunks
W3 = 3                # class waves: w0 = c0-3, w1 = c4-7, w2 = c8-9


def wave_ncls(w):
    return 4 if w < 2 else 2


def build_nc(debug=False):
    nc = bacc.Bacc("TRN2", target_bir_lowering=False, debug=debug)

    xtr_d = nc.declare_dram_parameter("xtr", [128, Q, I, BL], F16, isOutput=False)
    wfrp_d = nc.declare_dram_parameter("wfrp", [128, Q, I, PW], F16, isOutput=False)
    w2a_d = nc.declare_dram_parameter("w2a", [128, 2, RI], F16, isOutput=False)
    w2b_d = nc.declare_dram_parameter("w2b", [64, RI], F16, isOutput=False)
    xrep_d = nc.declare_dram_parameter("xrep", [128, RI], F16, isOutput=False)
    ident_d = nc.declare_dram_parameter("ident", [128, 128], F32, isOutput=False)
    out_d = nc.declare_dram_parameter("out", [C, O, BL], F32, isOutput=True)

    with tile.TileContext(nc) as tc, ExitStack() as ctx:
        res = ctx.enter_context(tc.tile_pool(name="res", bufs=1))
        yp = ctx.enter_context(tc.tile_pool(name="yp", bufs=2))
        gmp = ctx.enter_context(tc.tile_pool(name="gmp", bufs=2))
        trp = ctx.enter_context(tc.tile_pool(name="trp", bufs=2))
        cwp = ctx.enter_context(tc.tile_pool(name="cwp", bufs=1))
        smp = ctx.enter_context(tc.tile_pool(name="smp", bufs=1))
        psS = ctx.enter_context(
            tc.tile_pool(name="psS", bufs=2, space=bass.MemorySpace.PSUM)
        )
        psG = ctx.enter_context(
            tc.tile_pool(name="psG", bufs=2, space=bass.MemorySpace.PSUM)
        )
        psT = ctx.enter_context(
            tc.tile_pool(name="psT", bufs=1, space=bass.MemorySpace.PSUM)
        )
        psN = ctx.enter_context(
            tc.tile_pool(name="psN", bufs=1, space=bass.MemorySpace.PSUM)
        )

        # ---- resident tensors -------------------------------------
        xtr = res.tile([128, Q, I, BL], F16)
        wfrp = res.tile([128, Q, I, PW], F16)
        w2a = res.tile([128, 2, RI], F16)
        w2b = res.tile([64, RI], F16)
        xrep = res.tile([128, RI], F16)
        ident = res.tile([128, 128], F32)
        L = res.tile([128, W3, R], F32)
        S = res.tile([128, W3, 128], F16)    # agreement stationaries
        cwT = res.tile([128, Q, CB], F16)    # softmax weights, [rr, q, 32c+b]
        sps = res.tile([16, C, BL], F32)     # gathered s, [o, c, b]
        v_sb = res.tile([16, C, BL], F32)    # squash output, [o, c, b]
        Z = res.tile([128, W3], F32)
        Zi = res.tile([128, W3], F32)
        ones16 = res.tile([16, 1], F32)

        # ---- input DMAs (ordered to overlap with it0 compute) -----
        nc.sync.dma_start(ident[:], ident_d[:])
        nc.sync.dma_start(xtr[:], xtr_d[:])
        for q in range(Q):
            nc.sync.dma_start(wfrp[:, q], wfrp_d[:, q])
        nc.sync.dma_start(xrep[:], xrep_d[:])
        SEG = RI // 3
        for w in range(2):
            for s3 in range(3):
                nc.sync.dma_start(
                    w2a[:, w, s3 * SEG : (s3 + 1) * SEG],
                    w2a_d[:, w, s3 * SEG : (s3 + 1) * SEG],
                )
        for s3 in range(3):
            nc.sync.dma_start(
                w2b[:, s3 * SEG : (s3 + 1) * SEG],
                w2b_d[:, s3 * SEG : (s3 + 1) * SEG],
            )
        nc.vector.memset(S[:], 0.0)
        nc.vector.memset(ones16[:], 1.0)

        # ---------------------------------------------------------------
        def squash_chain(sn, f, n, it0=False):
            """f = squash scale from sn = |s|^2 (on [p, n] tiles)."""
            u1 = smp.tile(list(sn.shape), F32, tag="u1")
            u2 = smp.tile(list(sn.shape), F32, tag="u2")
            u3 = smp.tile(list(sn.shape), F32, tag="u3")
            if it0:
                nc.vector.tensor_scalar_mul(sn[:, :n], sn[:, :n], 1.0 / (R * R))
            nc.vector.tensor_scalar_add(u1[:, :n], sn[:, :n], EPS)
            nc.scalar.activation(
                u2[:, :n], u1[:, :n], mybir.ActivationFunctionType.Sqrt
            )
            nc.vector.tensor_scalar_add(u3[:, :n], sn[:, :n], 1.0)
            nc.vector.tensor_tensor(u1[:, :n], u2[:, :n], u3[:, :n], MUL)
            nc.vector.reciprocal(u2[:, :n], u1[:, :n])
            nc.vector.tensor_tensor(f[:, :n], sn[:, :n], u2[:, :n], MUL)
            if it0:
                nc.vector.tensor_scalar_mul(f[:, :n], f[:, :n], 1.0 / R)

        # ---------------------------------------------------------------
        # it0: s = sum over (r,i) of W * x (uniform weights folded into
        # the squash scalars).  One matmul per (q,i): 320-col moving.
        ps0 = psS.tile([128, PW], F32, tag="spsB")
        for q in range(Q):
            for i in range(I):
                nc.tensor.matmul(
                    ps0[:BL, :],
                    xtr[:, q, i, :],
                    wfrp[:, q, i, :],
                    start=(q == 0 and i == 0),
                    stop=(q == Q - 1 and i == I - 1),
                )
        sq0 = smp.tile([BL, C, 32], F32, tag="sq0")
        nc.scalar.activation(
            sq0[:],
            ps0[:BL, :].rearrange("b (c s) -> b c s", s=32),
            mybir.ActivationFunctionType.Square,
        )
        sn0 = smp.tile([BL, C], F32, tag="sn0")
        nc.vector.tensor_reduce(sn0[:], sq0[:], AXX, ADD)
        f0 = smp.tile([BL, C], F32, tag="f0")
        squash_chain(sn0, f0, C, it0=True)
        v0 = smp.tile([BL, PW], F32, tag="v0")
        nc.vector.tensor_tensor(
            v0[:].rearrange("b (c s) -> b c s", s=32),
            ps0[:BL, :].rearrange("b (c s) -> b c s", s=32),
            f0[:].unsqueeze(2).broadcast_to([BL, C, 32]),
            MUL,
        )
        vT = smp.tile([BL, PW], F32, tag="vT")
        nc.vector.transpose(vT[:], v0[:])    # 32x32 blocks: [o, 32c+b]
        for c in range(C):
            w, k = (c // 4, c % 4) if c < 8 else (2, c - 8)
            nc.scalar.copy(
                S[32 * k : 32 * k + 16, w, 32 * k : 32 * k + 32],
                vT[0:16, 32 * c : 32 * c + 32],
            )

        # ---------------------------------------------------------------
        def agreement_wave(w, acc):
            """L[p=(k,b), w, r] (+)= sum_{i,o} v*W*x for wave w."""
            npart = 128 if w < 2 else 64
            for n0 in range(NG):
                off = n0 * GCH
                gps = psG.tile([128, GCH], F32, tag="gps")
                for sub in range(0, GCH, 512):
                    if w < 2:
                        nc.tensor.matmul(
                            gps[:, sub : sub + 512],
                            S[:, w, :],
                            w2a[:, w, off + sub : off + sub + 512],
                            start=True,
                            stop=True,
                        )
                    else:
                        nc.tensor.matmul(
                            gps[:64, sub : sub + 512],
                            S[:64, 2, :64],
                            w2b[:, off + sub : off + sub + 512],
                            start=True,
                            stop=True,
                        )
                Lo = L[:npart, w, 128 * n0 : 128 * (n0 + 1)]
                if n0 % 2 == 1:
                    # GPSIMD unit (cannot read PSUM): scalar drains the
                    # bank to fp16 SBUF, GPSIMD does multiply + tree.
                    gsb = gmp.tile([128, GCH], F16, tag="gsb")
                    nc.scalar.copy(gsb[:npart], gps[:npart])
                    gm = gmp.tile([128, GCH], F16, tag="gmg")
                    nc.gpsimd.tensor_tensor(
                        gm[:npart], gsb[:npart], xrep[:npart, off : off + GCH], MUL
                    )
                    gmv = gm.rearrange("p (r i) -> p r i", i=I)
                    l1 = trp.tile([128, GCH // 2], F16, tag="l1")
                    l2 = trp.tile([128, GCH // 4], F16, tag="l2")
                    l1v = l1.rearrange("p (r i) -> p r i", i=4)
                    l2v = l2.rearrange("p (r i) -> p r i", i=2)
                    nc.gpsimd.tensor_tensor(
                        l1v[:npart], gmv[:npart, :, 0:4], gmv[:npart, :, 4:8], ADD
                    )
                    nc.gpsimd.tensor_tensor(
                        l2v[:npart], l1v[:npart, :, 0:2], l1v[:npart, :, 2:4], ADD
                    )
                    if not acc:
                        nc.gpsimd.tensor_tensor(
                            Lo, l2v[:npart, :, 0], l2v[:npart, :, 1], ADD
                        )
                    else:
                        a = trp.tile([128, GCH // 8], F32, tag="a")
                        nc.gpsimd.tensor_tensor(
                            a[:npart], l2v[:npart, :, 0], l2v[:npart, :, 1], ADD
                        )
                        nc.gpsimd.tensor_tensor(Lo, Lo, a[:npart], ADD)
                else:
                    gm = gmp.tile([128, GCH], F16, tag="gm")
                    nc.vector.tensor_tensor(
                        gm[:npart], gps[:npart], xrep[:npart, off : off + GCH], MUL
                    )
                    gmv = gm[:npart].rearrange("p (r i) -> p r i", i=I)
                    if not acc:
                        nc.vector.tensor_reduce(Lo, gmv, AXX, ADD)
                    else:
                        red = trp.tile([128, 128], F32, tag="red")
                        nc.vector.tensor_reduce(red[:npart], gmv, AXX, ADD)
                        nc.vector.tensor_tensor(Lo, Lo, red[:npart], ADD)

        # ---------------------------------------------------------------
        def softmax_wave(w):
            """cw = softmax_r(L[.,w,.]); transposed fp16 into cwT."""
            npart = 128 if w < 2 else 64
            cwv = cwp.tile([128, R], F32, tag=f"cw{w}")
            nc.scalar.activation(
                cwv[:npart],
                L[:npart, w, :],
                mybir.ActivationFunctionType.Exp,
                accum_out=Z[:npart, w : w + 1],
            )
            nc.vector.reciprocal(Zi[:npart, w : w + 1], Z[:npart, w : w + 1])
            nc.vector.tensor_scalar_mul(
                cwv[:npart], cwv[:npart], Zi[:npart, w : w + 1]
            )
            for q in range(Q):
                tps = psT.tile([128, 128], F32, tag="tps")
                nc.tensor.transpose(
                    tps[:, :npart],
                    cwv[:npart, 128 * q : 128 * (q + 1)],
                    ident[:npart, :npart],
                )
                nc.scalar.copy(cwT[:, q, 128 * w : 128 * w + npart], tps[:, :npart])

        # ---------------------------------------------------------------
        def spass_wave(w):
            """s[4w+cc, o, b] = sum_{r,i} W * cw * x for wave w."""
            ncls = wave_ncls(w)
            nw = 32 * ncls
            spsB = psS.tile([128, PW], F32, tag="spsB")
            for q in range(Q):
                y_q = yp.tile([128, I, 128], F16, tag="y")
                eng = nc.gpsimd if q % 3 == 2 else nc.vector
                eng.tensor_tensor(
                    y_q[:, :, :nw].rearrange("p i (c b) -> p i c b", b=BL),
                    xtr[:, q, :, :]
                    .unsqueeze(2)
                    .broadcast_to([128, I, ncls, BL]),
                    cwT[:, q, 128 * w : 128 * w + nw]
                    .rearrange("p (c b) -> p c b", b=BL)
                    .unsqueeze(1)
                    .broadcast_to([128, I, ncls, BL]),
                    MUL,
                )
                for i in range(I):
                    nc.tensor.matmul(
                        spsB[:nw, :nw],
                        wfrp[:, q, i, 128 * w : 128 * w + nw],
                        y_q[:, i, :nw],
                        start=(q == 0 and i == 0),
                        stop=(q == Q - 1 and i == I - 1),
                    )
            for cc in range(ncls):
                nc.scalar.copy(
                    sps[:, 4 * w + cc, :],
                    spsB[32 * cc : 32 * cc + 16, 32 * cc : 32 * cc + 32],
                )

        def squash_round():
            """v_sb = squash(s) over o, batched for all 10 classes."""
            spw = sps[:].rearrange("o c b -> o (c b)")
            sq = smp.tile([16, CB], F32, tag="sq")
            nc.vector.tensor_tensor(sq[:], spw, spw, MUL)
            snps = psN.tile([1, CB], F32, tag="snps")
            nc.tensor.matmul(snps[:], ones16[:], sq[:], start=True, stop=True)
            sn = smp.tile([1, CB], F32, tag="sn")
            nc.vector.tensor_copy(sn[:], snps[:])
            f = smp.tile([1, CB], F32, tag="f")
            squash_chain(sn, f, CB)
            frep = smp.tile([16, CB], F32, tag="frep")
            nc.gpsimd.partition_broadcast(frep[:], f[:])
            nc.vector.tensor_tensor(
                v_sb[:].rearrange("o c b -> o (c b)"), spw, frep[:], MUL
            )

        def fill_S_all():
            for c in range(C):
                w, k = (c // 4, c % 4) if c < 8 else (2, c - 8)
                nc.scalar.copy(
                    S[32 * k : 32 * k + 16, w, 32 * k : 32 * k + 32],
                    v_sb[:, c, :],
                )

        # =========================== flow ==============================
        for it in (1, 2):
            for w in range(W3):
                agreement_wave(w, acc=(it == 2))
                softmax_wave(w)
                spass_wave(w)
            squash_round()
            if it < 2:
                fill_S_all()

        nc.sync.dma_start(out_d[:].rearrange("c o b -> o c b"), v_sb[:])

    nc.compile()
    return nc


# =================== host-side prep / entry point =====================

def _prep_shared(W):
    """Per-problem constant tensors (replicated on every core)."""
    W = np.ascontiguousarray(W, np.float32)
    # wfrp[rr, q, i, 32c + o] = W[c, 128q+rr, i, o] (16-col zero pad per class)
    wfrp = np.zeros((128, Q, I, PW), np.float16)
    wr = W.reshape(C, Q, 128, I, O).transpose(2, 1, 3, 0, 4)  # [rr,q,i,c,o]
    for c in range(C):
        wfrp[:, :, :, 32 * c : 32 * c + O] = wr[:, :, :, c]
    # w2a[32k+o, w, 8r+i] = W[4w+k, r, i, o];  w2b[32k+o, 8r+i] = W[8+k,...]
    w2a = np.zeros((128, 2, RI), np.float16)
    for w in range(2):
        for k in range(4):
            w2a[32 * k : 32 * k + 16, w, :] = (
                W[4 * w + k].transpose(2, 0, 1).reshape(O, RI)
            )
    w2b = np.zeros((64, RI), np.float16)
    for k in range(2):
        w2b[32 * k : 32 * k + 16, :] = W[8 + k].transpose(2, 0, 1).reshape(O, RI)
    ident = np.eye(128, dtype=np.float32)
    return wfrp, w2a, w2b, ident


def _prep_core(x_shard):
    """Per-core tensors for one 32-batch shard: xtr and xrep."""
    xs = np.ascontiguousarray(x_shard, np.float32)       # [32, 1152, 8]
    xtr = np.ascontiguousarray(
        xs.reshape(BL, Q, 128, I).transpose(2, 1, 3, 0)
    ).astype(np.float16)                                  # [128, Q, I, 32]
    flat = xs.reshape(BL, RI)                             # [b, 8r+i]
    xrep = np.ascontiguousarray(
        flat[np.arange(128) % BL].astype(np.float16)
    )                                                     # [128, RI]
    return xtr, xrep


def prep_in_maps(x, W):
    wfrp, w2a, w2b, ident = _prep_shared(W)
    in_maps = []
    for m in range(NC):
        xtr, xrep = _prep_core(x[m * BL : (m + 1) * BL])
        in_maps.append(
            {
                "xtr": xtr,
                "wfrp": wfrp,
                "w2a": w2a,
                "w2b": w2b,
                "xrep": xrep,
                "ident": ident,
            }
        )
    return in_maps


_NC_CACHE = {}


def kernel(x, W):
    x = np.asarray(x, np.float32)
    W = np.asarray(W, np.float32)
    if "nc" not in _NC_CACHE:
        _NC_CACHE["nc"] = build_nc()
    nc = _NC_CACHE["nc"]

    res = run_bass_kernel_spmd(nc, prep_in_maps(x, W), list(range(NC)))
    out = np.empty((C, B, 1, 1, O), np.float32)
    for m in range(NC):
        o = res.results[m]["out"]                         # [C, O, BL]
        out[:, m * BL : (m + 1) * BL, 0, 0, :] = np.asarray(o).transpose(0, 2, 1)
    return out


if __name__ == "__main__":
    d = np.load("/root/problem/ref_data.npz")
    got = kernel(d["x"], d["W"])
    exp = d["expected"]
    err = np.abs(got - exp).max() / np.abs(exp).max()
    print("Relative error:", err)
